# revision 1
# baseline (speedup 1.0000x reference)
"""ChebyKAN layer on 8 TRN2 NeuronCores (data-parallel over batch).

y[b,o] = sum_{i,d} T_d(tanh(x[b,i])) * C[i,o,d],  d = 0..8

Device algorithm (per core, batch shard of 2048 rows, blocks of 512):
  - T_0 = 1 is folded into a host-computed bias; a K=1 ones x bias matmul
    seeds each PSUM accumulation group with it.
  - t = tanh(x) on ACT (basis laid out transposed: [i_chunk=128 part, batch
    free], two i-chunks batched per op -> [128, 1024] tiles).
  - Chebyshev basis in fp32 via product identities split across engines:
    DVE:    T2 = 2t^2-1, T4 = 2T2^2-1, T6 = 2T2*T4-T2, T8 = 2T2*T6-T4
    GpSimd: T3 = t*(2T2-1), T5 = (2T2)*T3 - t, T7 = (2T2)*T5 - T3
  - Basis rounded fp32 -> fp16 in two wide ACT casts (stage A: t,T2,T3,T4;
    stage B: T5..T8); fp16 keeps 11 mantissa bits (like f32r) but the
    2-byte LDWEIGHTS hides under the matmuls, unlike 4-byte f32r.
  - PE: stationary = fp16 basis slice [128,128], moving = fp16 coefficient
    chunk [i=128, o=512], accumulated over (d, i_chunk) into PSUM
    [b=128, o=512]; coefficients are gpsimd cast-DMA'd fp32 -> fp16 once
    and stay resident (4.2 MB).
  - PSUM evacuated with an ACT copy, stored to DRAM over sync-engine DMA.

Measured on trn2 (8 cores, NTFF profile): ~202 us HW exec, relative error
~2.5e-4 vs the fp32 jax reference (fp16 rounding of basis + coefficients).

Inputs arrive FULL; sharding/transpose/reorder happen on the host here.
"""

import numpy as np

import concourse.bacc as bacc
import concourse.tile as tile
from concourse import mybir
from concourse.bass_utils import run_bass_kernel_spmd

dt = mybir.dt

BATCH = 16384
I_DIM = 512
O_DIM = 512
DEG = 8            # d = 1..8 on device; d=0 via bias
N_CORES = 8
B_CORE = BATCH // N_CORES      # 2048
B_BLK = 512                    # batch rows per block
N_BLK = B_CORE // B_BLK        # 4
N_IC = I_DIM // 128            # 4 input chunks
N_BS = B_BLK // 128            # 4 psum row-tiles per block

_CACHE = {}


def _build_program():
    from contextlib import ExitStack

    AF = mybir.ActivationFunctionType
    OP = mybir.AluOpType

    nc = bacc.Bacc(num_swdge_queues=4)
    xt_in = nc.declare_dram_parameter("xt", [I_DIM, B_CORE], dt.float32, isOutput=False)
    cd_in = nc.declare_dram_parameter("cd", [DEG, I_DIM, O_DIM], dt.float32, isOutput=False)
    bias_in = nc.declare_dram_parameter("bias", [1, O_DIM], dt.float32, isOutput=False)
    ones_in = nc.declare_dram_parameter("ones", [1, 128], dt.float32, isOutput=False)
    y_out = nc.declare_dram_parameter("y", [B_CORE, O_DIM], dt.float32, isOutput=True)

    # Two i-chunks are batched per elementwise op: every chain op works
    # on [128, 2*B_BLK] = [128, 1024].  Degrees live in slots:
    #   stage A slots: 0:t 1:T2 2:T3 3:T4   -> cast A
    #   stage B slots: 0:T5 1:T6 2:T7 3:T8  -> cast B
    PW = 2 * B_BLK            # 1024, pair width
    DEG_A = {1: 0, 2: 1, 3: 2, 4: 3}
    DEG_B = {5: 0, 6: 1, 7: 2, 8: 3}

    with tile.TileContext(nc) as tc, ExitStack() as ctx:
        cpool = ctx.enter_context(tc.tile_pool(name="cpool", bufs=1))
        xpool = ctx.enter_context(tc.tile_pool(name="xpool", bufs=2))
        fpool = ctx.enter_context(tc.tile_pool(name="fpool", bufs=2))
        rpool = ctx.enter_context(tc.tile_pool(name="rpool", bufs=2))
        mvpool = ctx.enter_context(tc.tile_pool(name="mvpool", bufs=2))
        s2pool = ctx.enter_context(tc.tile_pool(name="s2pool", bufs=1))
        mgpool = ctx.enter_context(tc.tile_pool(name="mgpool", bufs=2))
        opool = ctx.enter_context(tc.tile_pool(name="opool", bufs=2))
        pspool = ctx.enter_context(tc.tile_pool(name="pspool", bufs=8, space="PSUM"))

        # Bias (T_0 term) and a ones row: K=1 matmul seeds PSUM with the bias.
        bias_t = cpool.tile([1, O_DIM], dt.float16, tag="bias")
        nc.gpsimd.dma_start(out=bias_t[:], in_=bias_in[:])
        ones_t = cpool.tile([1, 128], dt.float16, tag="ones")
        nc.gpsimd.dma_start(out=ones_t[:], in_=ones_in[:])

        # Coefficients: one wide cast-DMA (fp32 -> f32r) per degree, resident.
        c_tiles = {}
        for d in range(DEG):
            c = cpool.tile([128, N_IC, O_DIM], dt.float16, tag=f"c{d}", name=f"c{d}")
            nc.gpsimd.dma_start(
                out=c[:],
                in_=cd_in[d].rearrange("(ic p) o -> p ic o", p=128),
            )
            c_tiles[d] = c

        for blk in range(N_BLK):
            b0 = blk * B_BLK
            ps = []
            for bs in range(N_BS):
                p = pspool.tile([128, O_DIM], dt.float32, tag="ps", name="ps")
                nc.tensor.matmul(
                    p[:], lhsT=ones_t[:], rhs=bias_t[:], start=True, stop=False
                )
                ps.append(p)
            for pair in range(N_IC // 2):
                ic0 = pair * 2
                xt = xpool.tile([128, PW], dt.float32, tag="xt")
                for h in range(2):
                    ic = ic0 + h
                    nc.sync.dma_start(
                        out=xt[:, h * B_BLK:(h + 1) * B_BLK],
                        in_=xt_in[ic * 128:(ic + 1) * 128, b0:b0 + B_BLK],
                    )
                FA = fpool.tile([128, 4 * PW], dt.float32, tag="FA", name="FA")
                FB = fpool.tile([128, 4 * PW], dt.float32, tag="FB", name="FB")

                def sa(i):
                    return FA[:, i * PW:(i + 1) * PW]

                def sb(i):
                    return FB[:, i * PW:(i + 1) * PW]

                t, s, T3, T4 = sa(0), sa(1), sa(2), sa(3)
                T5, T6, T7, T8 = sb(0), sb(1), sb(2), sb(3)

                nc.scalar.activation(t, xt[:], AF.Tanh)

                # DVE: T2, preps, even chain (tensor_scalar runs in 2x mode)
                m2 = mvpool.tile([128, PW], dt.float32, tag="mv", name="m2")
                nc.vector.scalar_tensor_tensor(m2[:], t, 2.0, t, OP.mult, OP.mult)
                nc.vector.tensor_scalar_sub(s, m2[:], 1.0)
                s2 = s2pool.tile([128, PW], dt.float32, tag="s2", name="s2")
                nc.vector.tensor_scalar_mul(s2[:], s, 2.0)
                w = s2pool.tile([128, PW], dt.float32, tag="w", name="w")
                nc.vector.tensor_scalar(w[:], s, 2.0, 1.0, OP.mult, OP.subtract)
                m4 = mvpool.tile([128, PW], dt.float32, tag="mv", name="m4")
                nc.vector.scalar_tensor_tensor(m4[:], s, 2.0, s, OP.mult, OP.mult)
                nc.vector.tensor_scalar_sub(T4, m4[:], 1.0)
                m6 = mvpool.tile([128, PW], dt.float32, tag="mv", name="m6")
                nc.vector.scalar_tensor_tensor(m6[:], T4, 2.0, s, OP.mult, OP.mult)
                nc.vector.tensor_sub(T6, m6[:], s)
                m8 = mvpool.tile([128, PW], dt.float32, tag="mv", name="m8")
                nc.vector.scalar_tensor_tensor(m8[:], T6, 2.0, s, OP.mult, OP.mult)
                nc.vector.tensor_sub(T8, m8[:], T4)

                # GpSimd: odd chain muls; final T7 subtract on DVE
                nc.gpsimd.tensor_mul(T3, t, w[:])
                m5 = mgpool.tile([128, PW], dt.float32, tag="mg", name="m5")
                nc.gpsimd.tensor_mul(m5[:], s2[:], T3)
                nc.gpsimd.tensor_sub(T5, m5[:], t)
                m7 = mgpool.tile([128, PW], dt.float32, tag="mg", name="m7")
                nc.gpsimd.tensor_mul(m7[:], s2[:], T5)
                nc.gpsimd.tensor_sub(T7, m7[:], T3)

                # Two-stage rounding casts fp32 -> fp16 on ACT.
                RA = rpool.tile([128, 4 * PW], dt.float16, tag="RA", name="RA")
                nc.scalar.activation(RA[:], FA[:], AF.Copy)
                RB = rpool.tile([128, 4 * PW], dt.float16, tag="RB", name="RB")
                nc.scalar.activation(RB[:], FB[:], AF.Copy)

                # Matmuls: stage-A degrees first (overlap with cast B).
                for stage, R, degs in (("A", RA, DEG_A), ("B", RB, DEG_B)):
                    for h in range(2):
                        ic = ic0 + h
                        for bs in range(N_BS):
                            for d, slot in degs.items():
                                nc.tensor.matmul(
                                    ps[bs][:],
                                    lhsT=R[:, slot * PW + h * B_BLK + bs * 128:
                                           slot * PW + h * B_BLK + (bs + 1) * 128],
                                    rhs=c_tiles[d - 1][:, ic, :],
                                    start=False,
                                    stop=(pair == 1 and stage == "B"
                                          and h == 1 and d == DEG),
                                )

            for bs in range(N_BS):
                o = opool.tile([128, O_DIM], dt.float32, tag="o")
                nc.scalar.activation(o[:], ps[bs][:], AF.Copy)
                nc.sync.dma_start(
                    out=y_out[b0 + bs * 128: b0 + (bs + 1) * 128, :], in_=o[:]
                )

    nc.compile()
    return nc


def _get_program():
    if "nc" not in _CACHE:
        _CACHE["nc"] = _build_program()
    return _CACHE["nc"]


def _prep_inputs(x, cheby_coeffs):
    x = np.ascontiguousarray(x, dtype=np.float32)
    c = np.ascontiguousarray(cheby_coeffs, dtype=np.float32)
    cd = np.ascontiguousarray(np.transpose(c, (2, 0, 1))[1:DEG + 1])  # [8, I, O]
    bias = c[:, :, 0].sum(axis=0, dtype=np.float64).astype(np.float32)[None, :]
    ones = np.ones((1, 128), dtype=np.float32)
    in_maps = []
    for core in range(N_CORES):
        xs = x[core * B_CORE:(core + 1) * B_CORE]          # [2048, I]
        xt = np.ascontiguousarray(xs.T)                     # [I, 2048]
        in_maps.append({"xt": xt, "cd": cd, "bias": bias, "ones": ones})
    return in_maps


def run(x, cheby_coeffs, trace=False, **trace_kwargs):
    nc = _get_program()
    in_maps = _prep_inputs(x, cheby_coeffs)
    res = run_bass_kernel_spmd(
        nc, in_maps, list(range(N_CORES)), trace=trace, **trace_kwargs
    )
    y = np.concatenate([res.results[i]["y"] for i in range(N_CORES)], axis=0)
    return y, res


def kernel(x, cheby_coeffs):
    y, _ = run(x, cheby_coeffs)
    return y



# revision 4
# speedup vs baseline: 1.4240x; 1.4240x over previous
"""ChebyKAN layer on 8 TRN2 NeuronCores (data-parallel over batch).

y[b,o] = sum_{i,d} T_d(tanh(x[b,i])) * C[i,o,d],  d = 0..8

Key idea vs the DVE-recurrence baseline: almost the whole Chebyshev basis
is built on the ACT engine with Square ops (1-ULP, present in every ACT
table set -> a single table load), using affine pre-scale folding:

    t   = tanh(x)                     (ACT Tanh)
    q2  = (sqrt2*t)^2        = 2t^2          = T2 + 1
    q4  = (sqrt2*q2-sqrt2)^2 = 2(q2-1)^2     = T4 + 1
    q8  = (sqrt2*q4-sqrt2)^2 = 2(q4-1)^2     = T8 + 1
    r5  = (2*q2-2.5)^2       = 16t^4-20t^2+6.25

The PE consumes 8 basis rows per i-chunk; affine shifts between these
rows and true Chebyshev polynomials are folded into host-side
coefficient/bias transforms (all linear):

    row0 = t          -> C1' = C1 - 3*C3 - 1.25*C5 - C7
    row1 = T2 = q2-1  -> C2
    row2 = m3 = 4t^3  -> C3          (T3 = m3 - 3t)
    row3 = B5 = r5*t  -> C5          (T5 = B5 - 1.25t)
    row4 = B6 = 2T3^2 -> C6          (T6 = B6 - 1)
    row5 = T4 = q4-1  -> C4
    row6 = B7 = 2T4T3 -> C7          (T7 = B7 - t)
    row7 = q8         -> C8          (T8 = q8 - 1)
    bias' = sum_i (C0 - C6 - C8)     (T0 term + the shifted rows)

Per core (batch shard 2048 rows) the work is 4 "quarters" of 512 rows,
each mapping 1:1 onto a PSUM accumulation group of 4 banks; basis tiles
are [i=128 part, b=512 free] fp16, double-buffered so ACT/DVE production
of quarter q+1 overlaps PE consumption of quarter q. DVE does 7 cheap
fp16 products per (quarter, i-chunk); GpSimd evacuates PSUM.

Inputs arrive FULL; sharding/transpose/folding happen on the host here.
"""

import numpy as np

import concourse.bacc as bacc
import concourse.tile as tile
from concourse import mybir
from concourse.bass_utils import run_bass_kernel_spmd

dt = mybir.dt

BATCH = 16384
I_DIM = 512
O_DIM = 512
N_CORES = 8
B_CORE = BATCH // N_CORES      # 2048
QW = 512                       # quarter width (psum group rows)
N_Q = B_CORE // QW             # 4
N_IC = I_DIM // 128            # 4
N_BS = QW // 128               # 4
SQ2 = float(np.float32(np.sqrt(2.0)))

_CACHE = {}


def _build_program():
    from contextlib import ExitStack

    AF = mybir.ActivationFunctionType
    OP = mybir.AluOpType

    nc = bacc.Bacc(num_swdge_queues=4)
    xt_in = nc.declare_dram_parameter("xt", [I_DIM, B_CORE], dt.float32, isOutput=False)
    cd_in = nc.declare_dram_parameter("cd", [8, I_DIM, O_DIM], dt.float32, isOutput=False)
    bias_in = nc.declare_dram_parameter("bias", [1, O_DIM], dt.float32, isOutput=False)
    ones_in = nc.declare_dram_parameter("ones", [1, 128], dt.float32, isOutput=False)
    y_out = nc.declare_dram_parameter("y", [B_CORE, O_DIM], dt.float32, isOutput=True)

    with tile.TileContext(nc) as tc, ExitStack() as ctx:
        cpool = ctx.enter_context(tc.tile_pool(name="cpool", bufs=1))
        xpool = ctx.enter_context(tc.tile_pool(name="xpool", bufs=2))
        fpool = ctx.enter_context(tc.tile_pool(name="fpool", bufs=2))
        bpool = ctx.enter_context(tc.tile_pool(name="bpool", bufs=2))
        tpool = ctx.enter_context(tc.tile_pool(name="tpool", bufs=2))
        epool = ctx.enter_context(tc.tile_pool(name="epool", bufs=4))
        pspool = ctx.enter_context(tc.tile_pool(name="pspool", bufs=8, space="PSUM"))

        # Bias (T_0 + shifted-row corrections) seeded via K=1 matmul.
        bias_t = cpool.tile([1, O_DIM], dt.float16, tag="bias")
        nc.gpsimd.dma_start(out=bias_t[:], in_=bias_in[:])
        ones_t = cpool.tile([1, 128], dt.float16, tag="ones")
        nc.gpsimd.dma_start(out=ones_t[:], in_=ones_in[:])

        # Const bias columns for ACT Square affine pre-scale.
        nsq2 = cpool.tile([128, 1], dt.float32, tag="nsq2")
        nc.gpsimd.memset(nsq2[:], -SQ2)
        n2p5 = cpool.tile([128, 1], dt.float32, tag="n2p5")
        nc.gpsimd.memset(n2p5[:], -2.5)

        # Coefficients (host pre-folded/ordered): cast-DMA fp32 -> fp16.
        c_tiles = []
        for r in range(8):
            c = cpool.tile([128, N_IC, O_DIM], dt.float16, tag=f"c{r}", name=f"c{r}")
            nc.gpsimd.dma_start(
                out=c[:],
                in_=cd_in[r].rearrange("(ic p) o -> p ic o", p=128),
            )
            c_tiles.append(c)

        for q in range(N_Q):
            b0 = q * QW

            # ---- basis production (type-major across i-chunks) ----
            xts = []
            for ic in range(N_IC):
                xt = xpool.tile([128, QW], dt.float32, tag=f"x{ic}")
                nc.sync.dma_start(
                    out=xt[:], in_=xt_in[ic * 128:(ic + 1) * 128, b0:b0 + QW]
                )
                xts.append(xt)

            rows = [[None] * N_IC for _ in range(8)]

            # ACT stream (one table set: tanh + square live together)
            t16s, t32s, q2s, r5s, q4s = [], [], [], [], []
            for ic in range(N_IC):
                t16 = bpool.tile([128, QW], dt.float16, tag=f"b0_{ic}", name="t16")
                nc.scalar.activation(t16[:], xts[ic][:], AF.Tanh)
                rows[0][ic] = t16
                t16s.append(t16)
            for ic in range(N_IC):
                t32 = fpool.tile([128, QW], dt.float32, tag=f"t32_{ic}", name="t32")
                nc.scalar.activation(t32[:], xts[ic][:], AF.Tanh)
                t32s.append(t32)
            for ic in range(N_IC):
                q2 = fpool.tile([128, QW], dt.float32, tag=f"q2_{ic}", name="q2")
                nc.scalar.activation(q2[:], t32s[ic][:], AF.Square, scale=SQ2)
                q2s.append(q2)
            for ic in range(N_IC):
                r5 = tpool.tile([128, QW], dt.float16, tag=f"r5_{ic}", name="r5")
                nc.scalar.activation(
                    r5[:], q2s[ic][:], AF.Square, bias=n2p5[:], scale=2.0
                )
                r5s.append(r5)
            for ic in range(N_IC):
                q4 = fpool.tile([128, QW], dt.float32, tag=f"q4_{ic}", name="q4")
                nc.scalar.activation(
                    q4[:], q2s[ic][:], AF.Square, bias=nsq2[:], scale=SQ2
                )
                q4s.append(q4)
            for ic in range(N_IC):
                q8 = bpool.tile([128, QW], dt.float16, tag=f"b7_{ic}", name="q8")
                nc.scalar.activation(
                    q8[:], q4s[ic][:], AF.Square, bias=nsq2[:], scale=SQ2
                )
                rows[7][ic] = q8

            # DVE stream (fp16 products; order matches ACT production)
            for ic in range(N_IC):
                T2 = bpool.tile([128, QW], dt.float16, tag=f"b1_{ic}", name="T2")
                nc.vector.tensor_scalar_sub(T2[:], q2s[ic][:], 1.0)
                rows[1][ic] = T2
            m3s = []
            for ic in range(N_IC):
                m3 = bpool.tile([128, QW], dt.float16, tag=f"b2_{ic}", name="m3")
                nc.vector.scalar_tensor_tensor(
                    m3[:], q2s[ic][:], 2.0, t32s[ic][:], OP.mult, OP.mult
                )
                rows[2][ic] = m3
                m3s.append(m3)
            T3s = []
            for ic in range(N_IC):
                T3 = tpool.tile([128, QW], dt.float16, tag=f"t3_{ic}", name="T3")
                nc.vector.scalar_tensor_tensor(
                    T3[:], t16s[ic][:], -3.0, m3s[ic][:], OP.mult, OP.add
                )
                T3s.append(T3)
            for ic in range(N_IC):
                B5 = bpool.tile([128, QW], dt.float16, tag=f"b3_{ic}", name="B5")
                nc.vector.tensor_mul(B5[:], r5s[ic][:], t16s[ic][:])
                rows[3][ic] = B5
            for ic in range(N_IC):
                B6 = bpool.tile([128, QW], dt.float16, tag=f"b4_{ic}", name="B6")
                nc.vector.scalar_tensor_tensor(
                    B6[:], T3s[ic][:], 2.0, T3s[ic][:], OP.mult, OP.mult
                )
                rows[4][ic] = B6
            T4s = []
            for ic in range(N_IC):
                T4 = bpool.tile([128, QW], dt.float16, tag=f"b5_{ic}", name="T4")
                nc.vector.tensor_scalar_sub(T4[:], q4s[ic][:], 1.0)
                rows[5][ic] = T4
                T4s.append(T4)
            for ic in range(N_IC):
                B7 = bpool.tile([128, QW], dt.float16, tag=f"b6_{ic}", name="B7")
                nc.vector.scalar_tensor_tensor(
                    B7[:], T4s[ic][:], 2.0, T3s[ic][:], OP.mult, OP.mult
                )
                rows[6][ic] = B7

            # ---- PE: seed with bias, accumulate 8 rows x 4 i-chunks ----
            ps = []
            for bs in range(N_BS):
                p = pspool.tile([128, O_DIM], dt.float32, tag="ps", name="ps")
                nc.tensor.matmul(
                    p[:], lhsT=ones_t[:], rhs=bias_t[:], start=True, stop=False
                )
                ps.append(p)
            for r in range(8):
                for ic in range(N_IC):
                    lt = rows[r][ic]
                    for bs in range(N_BS):
                        nc.tensor.matmul(
                            ps[bs][:],
                            lhsT=lt[:, bs * 128:(bs + 1) * 128],
                            rhs=c_tiles[r][:, ic, :],
                            start=False,
                            stop=(r == 7 and ic == N_IC - 1),
                        )

            # ---- evacuate PSUM (GpSimd) and store ----
            for bs in range(N_BS):
                e = epool.tile([128, O_DIM], dt.float32, tag="e")
                nc.vector.tensor_copy(e[:], ps[bs][:])
                nc.sync.dma_start(
                    out=y_out[b0 + bs * 128: b0 + (bs + 1) * 128, :], in_=e[:]
                )

    nc.compile()
    return nc


def _get_program():
    if "nc" not in _CACHE:
        _CACHE["nc"] = _build_program()
    return _CACHE["nc"]


def _prep_inputs(x, cheby_coeffs):
    x = np.ascontiguousarray(x, dtype=np.float32)
    C = np.asarray(cheby_coeffs, dtype=np.float64)  # [I, O, 9]
    # Host-side folding of affine shifts between device rows and T_d.
    c1p = C[:, :, 1] - 3.0 * C[:, :, 3] - 1.25 * C[:, :, 5] - C[:, :, 7]
    rows = [c1p, C[:, :, 2], C[:, :, 3], C[:, :, 5],
            C[:, :, 6], C[:, :, 4], C[:, :, 7], C[:, :, 8]]
    cd = np.ascontiguousarray(np.stack(rows, axis=0), dtype=np.float32)
    bias = (C[:, :, 0].sum(axis=0) - C[:, :, 6].sum(axis=0)
            - C[:, :, 8].sum(axis=0)).astype(np.float32)[None, :]
    ones = np.ones((1, 128), dtype=np.float32)
    in_maps = []
    for core in range(N_CORES):
        xs = x[core * B_CORE:(core + 1) * B_CORE]          # [2048, I]
        xt = np.ascontiguousarray(xs.T)                     # [I, 2048]
        in_maps.append({"xt": xt, "cd": cd, "bias": bias, "ones": ones})
    return in_maps


def run(x, cheby_coeffs, trace=False, **trace_kwargs):
    nc = _get_program()
    in_maps = _prep_inputs(x, cheby_coeffs)
    res = run_bass_kernel_spmd(
        nc, in_maps, list(range(N_CORES)), trace=trace, **trace_kwargs
    )
    y = np.concatenate([res.results[i]["y"] for i in range(N_CORES)], axis=0)
    return y, res


def kernel(x, cheby_coeffs):
    y, _ = run(x, cheby_coeffs)
    return y


# revision 8
# speedup vs baseline: 1.4614x; 1.0263x over previous
"""ChebyKAN layer on 8 TRN2 NeuronCores (data-parallel over batch).

y[b,o] = sum_{i,d} T_d(tanh(x[b,i])) * C[i,o,d],  d = 0..8

Key idea vs the DVE-recurrence baseline: almost the whole Chebyshev basis
is built on the ACT engine with Square ops (1-ULP, present in every ACT
table set -> a single table load), using affine pre-scale folding:

    t   = tanh(x)                     (ACT Tanh)
    q2  = (sqrt2*t)^2        = 2t^2          = T2 + 1
    q4  = (sqrt2*q2-sqrt2)^2 = 2(q2-1)^2     = T4 + 1
    q8  = (sqrt2*q4-sqrt2)^2 = 2(q4-1)^2     = T8 + 1
    r5  = (2*q2-2.5)^2       = 16t^4-20t^2+6.25

The PE consumes 8 basis rows per i-chunk; affine shifts between these
rows and true Chebyshev polynomials are folded into host-side
coefficient/bias transforms (all linear):

    row0 = t          -> C1' = C1 - 3*C3 - 1.25*C5 - C7
    row1 = T2 = q2-1  -> C2
    row2 = m3 = 4t^3  -> C3          (T3 = m3 - 3t)
    row3 = B5 = r5*t  -> C5          (T5 = B5 - 1.25t)
    row4 = B6 = 2T3^2 -> C6          (T6 = B6 - 1)
    row5 = T4 = q4-1  -> C4
    row6 = B7 = 2T4T3 -> C7          (T7 = B7 - t)
    row7 = q8         -> C8          (T8 = q8 - 1)
    bias' = sum_i (C0 - C6 - C8)     (T0 term + the shifted rows)

Per core (batch shard 2048 rows) the work is 4 "quarters" of 512 rows,
each mapping 1:1 onto a PSUM accumulation group of 4 banks; basis tiles
are [i=128 part, b=512 free] fp16, double-buffered so ACT/DVE production
of quarter q+1 overlaps PE consumption of quarter q. DVE does 7 cheap
fp16 products per (quarter, i-chunk); GpSimd evacuates PSUM.

Inputs arrive FULL; sharding/transpose/folding happen on the host here.
"""

import numpy as np

import concourse.bacc as bacc
import concourse.tile as tile
from concourse import mybir
from concourse.bass_utils import run_bass_kernel_spmd

dt = mybir.dt

BATCH = 16384
I_DIM = 512
O_DIM = 512
N_CORES = 8
B_CORE = BATCH // N_CORES      # 2048
QW = 512                       # quarter width (psum group rows)
N_Q = B_CORE // QW             # 4
N_IC = I_DIM // 128            # 4
N_BS = QW // 128               # 4
SQ2 = float(np.float32(np.sqrt(2.0)))

_CACHE = {}


def _build_program():
    from contextlib import ExitStack

    AF = mybir.ActivationFunctionType
    OP = mybir.AluOpType

    nc = bacc.Bacc(num_swdge_queues=4)
    xt_in = nc.declare_dram_parameter("xt", [I_DIM, B_CORE], dt.float32, isOutput=False)
    cd_in = nc.declare_dram_parameter("cd", [8, I_DIM, O_DIM], dt.float32, isOutput=False)
    bias_in = nc.declare_dram_parameter("bias", [1, O_DIM], dt.float32, isOutput=False)
    ones_in = nc.declare_dram_parameter("ones", [1, 128], dt.float32, isOutput=False)
    y_out = nc.declare_dram_parameter("y", [B_CORE, O_DIM], dt.float32, isOutput=True)

    with tile.TileContext(nc) as tc, ExitStack() as ctx:
        cpool = ctx.enter_context(tc.tile_pool(name="cpool", bufs=1))
        xpool = ctx.enter_context(tc.tile_pool(name="xpool", bufs=2))
        fpool = ctx.enter_context(tc.tile_pool(name="fpool", bufs=2))
        bpool = ctx.enter_context(tc.tile_pool(name="bpool", bufs=2))
        tpool = ctx.enter_context(tc.tile_pool(name="tpool", bufs=2))
        epool = ctx.enter_context(tc.tile_pool(name="epool", bufs=4))
        pspool = ctx.enter_context(tc.tile_pool(name="pspool", bufs=8, space="PSUM"))

        # Bias (T_0 + shifted-row corrections) seeded via K=1 matmul.
        bias_t = cpool.tile([1, O_DIM], dt.float16, tag="bias")
        nc.gpsimd.dma_start(out=bias_t[:], in_=bias_in[:])
        ones_t = cpool.tile([1, 128], dt.float16, tag="ones")
        nc.gpsimd.dma_start(out=ones_t[:], in_=ones_in[:])

        # Const bias columns for ACT Square affine pre-scale. Memset on DVE:
        # a gpsimd memset forces a ~3us Q7 drain that delays the coefficient
        # DMA triggers queued behind it.
        nsq2 = cpool.tile([128, 1], dt.float32, tag="nsq2")
        nc.vector.memset(nsq2[:], -SQ2)
        n2p5 = cpool.tile([128, 1], dt.float32, tag="n2p5")
        nc.vector.memset(n2p5[:], -2.5)

        # Coefficients (host pre-folded/ordered): cast-DMA fp32 -> fp16.
        c_tiles = []
        for r in range(8):
            c = cpool.tile([128, N_IC, O_DIM], dt.float16, tag=f"c{r}", name=f"c{r}")
            nc.gpsimd.dma_start(
                out=c[:],
                in_=cd_in[r].rearrange("(ic p) o -> p ic o", p=128),
            )
            c_tiles.append(c)

        # Bias broadcast [128, 512]: one K=1 matmul replicates the bias row
        # across partitions; evacuation then fuses the add, so the 4 PSUM
        # seeds per quarter (16 matmuls) are not needed.
        bps = pspool.tile([128, O_DIM], dt.float32, tag="ps", name="bps")
        nc.tensor.matmul(bps[:], lhsT=ones_t[:], rhs=bias_t[:], start=True, stop=True)
        bias_bc = cpool.tile([128, O_DIM], dt.float32, tag="bias_bc")
        nc.vector.tensor_copy(bias_bc[:], bps[:])

        for q in range(N_Q):
            b0 = q * QW

            # ---- basis production (type-major across i-chunks) ----
            xts = []
            for ic in range(N_IC):
                xt = xpool.tile([128, QW], dt.float32, tag=f"x{ic}")
                nc.sync.dma_start(
                    out=xt[:], in_=xt_in[ic * 128:(ic + 1) * 128, b0:b0 + QW]
                )
                xts.append(xt)

            rows = [[None] * N_IC for _ in range(8)]

            # ACT stream (one table set: tanh + square live together)
            t16s, t32s, q2s, r5s, q4s = [], [], [], [], []
            for ic in range(N_IC):
                t16 = bpool.tile([128, QW], dt.float16, tag=f"b0_{ic}", name="t16")
                nc.scalar.activation(t16[:], xts[ic][:], AF.Tanh)
                rows[0][ic] = t16
                t16s.append(t16)
            for ic in range(N_IC):
                t32 = fpool.tile([128, QW], dt.float32, tag=f"t32_{ic}", name="t32")
                nc.scalar.activation(t32[:], xts[ic][:], AF.Tanh)
                t32s.append(t32)
            for ic in range(N_IC):
                q2 = fpool.tile([128, QW], dt.float32, tag=f"q2_{ic}", name="q2")
                nc.scalar.activation(q2[:], t32s[ic][:], AF.Square, scale=SQ2)
                q2s.append(q2)
            for ic in range(N_IC):
                r5 = tpool.tile([128, QW], dt.float16, tag=f"r5_{ic}", name="r5")
                nc.scalar.activation(
                    r5[:], q2s[ic][:], AF.Square, bias=n2p5[:], scale=2.0
                )
                r5s.append(r5)
            for ic in range(N_IC):
                q4 = fpool.tile([128, QW], dt.float32, tag=f"q4_{ic}", name="q4")
                nc.scalar.activation(
                    q4[:], q2s[ic][:], AF.Square, bias=nsq2[:], scale=SQ2
                )
                q4s.append(q4)
            for ic in range(N_IC):
                q8 = bpool.tile([128, QW], dt.float16, tag=f"b7_{ic}", name="q8")
                nc.scalar.activation(
                    q8[:], q4s[ic][:], AF.Square, bias=nsq2[:], scale=SQ2
                )
                rows[7][ic] = q8

            # DVE stream (fp16 products; order matches ACT production)
            for ic in range(N_IC):
                T2 = bpool.tile([128, QW], dt.float16, tag=f"b1_{ic}", name="T2")
                nc.vector.tensor_scalar_sub(T2[:], q2s[ic][:], 1.0)
                rows[1][ic] = T2
            m3s = []
            for ic in range(N_IC):
                m3 = bpool.tile([128, QW], dt.float16, tag=f"b2_{ic}", name="m3")
                nc.vector.scalar_tensor_tensor(
                    m3[:], q2s[ic][:], 2.0, t32s[ic][:], OP.mult, OP.mult
                )
                rows[2][ic] = m3
                m3s.append(m3)
            T3s = []
            for ic in range(N_IC):
                T3 = tpool.tile([128, QW], dt.float16, tag=f"t3_{ic}", name="T3")
                nc.vector.scalar_tensor_tensor(
                    T3[:], t16s[ic][:], -3.0, m3s[ic][:], OP.mult, OP.add
                )
                T3s.append(T3)
            for ic in range(N_IC):
                B5 = bpool.tile([128, QW], dt.float16, tag=f"b3_{ic}", name="B5")
                nc.vector.tensor_mul(B5[:], r5s[ic][:], t16s[ic][:])
                rows[3][ic] = B5
            for ic in range(N_IC):
                B6 = bpool.tile([128, QW], dt.float16, tag=f"b4_{ic}", name="B6")
                nc.vector.scalar_tensor_tensor(
                    B6[:], T3s[ic][:], 2.0, T3s[ic][:], OP.mult, OP.mult
                )
                rows[4][ic] = B6
            T4s = []
            for ic in range(N_IC):
                T4 = bpool.tile([128, QW], dt.float16, tag=f"b5_{ic}", name="T4")
                nc.vector.tensor_scalar_sub(T4[:], q4s[ic][:], 1.0)
                rows[5][ic] = T4
                T4s.append(T4)
            for ic in range(N_IC):
                B7 = bpool.tile([128, QW], dt.float16, tag=f"b6_{ic}", name="B7")
                nc.vector.scalar_tensor_tensor(
                    B7[:], T4s[ic][:], 2.0, T3s[ic][:], OP.mult, OP.mult
                )
                rows[6][ic] = B7

            # ---- PE: accumulate 8 rows x 4 i-chunks per PSUM bank ----
            ps = []
            for bs in range(N_BS):
                p = pspool.tile([128, O_DIM], dt.float32, tag="ps", name="ps")
                ps.append(p)
            for r in range(8):
                for ic in range(N_IC):
                    lt = rows[r][ic]
                    for bs in range(N_BS):
                        nc.tensor.matmul(
                            ps[bs][:],
                            lhsT=lt[:, bs * 128:(bs + 1) * 128],
                            rhs=c_tiles[r][:, ic, :],
                            start=(r == 0 and ic == 0),
                            stop=(r == 7 and ic == N_IC - 1),
                        )

            # ---- evacuate PSUM (DVE, fusing the bias add) and store ----
            for bs in range(N_BS):
                e = epool.tile([128, O_DIM], dt.float32, tag="e")
                nc.vector.tensor_add(e[:], ps[bs][:], bias_bc[:])
                nc.sync.dma_start(
                    out=y_out[b0 + bs * 128: b0 + (bs + 1) * 128, :], in_=e[:]
                )

    nc.compile()
    return nc


def _get_program():
    if "nc" not in _CACHE:
        _CACHE["nc"] = _build_program()
    return _CACHE["nc"]


def _prep_inputs(x, cheby_coeffs):
    x = np.ascontiguousarray(x, dtype=np.float32)
    C = np.asarray(cheby_coeffs, dtype=np.float64)  # [I, O, 9]
    # Host-side folding of affine shifts between device rows and T_d.
    c1p = C[:, :, 1] - 3.0 * C[:, :, 3] - 1.25 * C[:, :, 5] - C[:, :, 7]
    rows = [c1p, C[:, :, 2], C[:, :, 3], C[:, :, 5],
            C[:, :, 6], C[:, :, 4], C[:, :, 7], C[:, :, 8]]
    cd = np.ascontiguousarray(np.stack(rows, axis=0), dtype=np.float32)
    bias = (C[:, :, 0].sum(axis=0) - C[:, :, 6].sum(axis=0)
            - C[:, :, 8].sum(axis=0)).astype(np.float32)[None, :]
    ones = np.ones((1, 128), dtype=np.float32)
    in_maps = []
    for core in range(N_CORES):
        xs = x[core * B_CORE:(core + 1) * B_CORE]          # [2048, I]
        xt = np.ascontiguousarray(xs.T)                     # [I, 2048]
        in_maps.append({"xt": xt, "cd": cd, "bias": bias, "ones": ones})
    return in_maps


def run(x, cheby_coeffs, trace=False, **trace_kwargs):
    nc = _get_program()
    in_maps = _prep_inputs(x, cheby_coeffs)
    res = run_bass_kernel_spmd(
        nc, in_maps, list(range(N_CORES)), trace=trace, **trace_kwargs
    )
    y = np.concatenate([res.results[i]["y"] for i in range(N_CORES)], axis=0)
    return y, res


def kernel(x, cheby_coeffs):
    y, _ = run(x, cheby_coeffs)
    return y
